# revision 19
# baseline (speedup 1.0000x reference)
"""Cross-attention kernel for 8 Trainium2 NeuronCores (SPMD).

Problem: B=4, T_q=T_kv=2048, Q_DIM=1024, KV_DIM=768, H=16, DK=64, fp32.
  q = q_tokens @ Wq.T ; k = kv_tokens @ Wk.T ; v = kv_tokens @ Wv.T
  out = softmax(q k^T / sqrt(DK)) v @ Wo.T

Sharding (8 cores): core c handles batch b=c//2 and head-group hg=c%2
(8 heads, 512 of the 1024 q-dims).  After attention, the pair (2b, 2b+1)
AllGathers the per-head-group attention outputs (one collective per
head-pair chunk, overlapped with the remaining attention work), then each
core runs the output projection against ITS half of the Wo columns —
core c returns out[b, :, (c%2)*512:(c%2+1)*512] transposed.  The
rank-dependent output-channel split lives entirely in the host-side Wo
slice, so the device program is identical on all cores.

On-device layout is channel-major ("transposed") end-to-end: all host
inputs are pre-transposed so every matmul contraction dim lands on SBUF
partitions with no device-side transposes.  Softmax runs without
max-subtraction (scores are O(6) for randn inputs; exp is safe in fp32)
and the denominator comes free from an appended ones-column in V during
the PV matmul.  All matmuls run as float32r (full PE rate at
moving-dim 512); attention score matmuls for the two heads of a pair
run concurrently in the two 64-row halves of the PE array (row tiling).
"""

import numpy as np

import concourse.bacc as bacc
import concourse.mybir as mybir
import concourse.tile as tile
from concourse import bass_utils

N_CORES = 8
P = 128
TQ = 2048
TKV = 2048
CQ = 1024     # q_tokens channels
CKV = 768     # kv_tokens channels
DQ = 512      # per-core head-group q dims (8 heads x 64)
DO = 512      # per-core output channels (half of 1024)
NJ = 4        # 512-wide t-blocks
NTB = 4       # projection t-blocks
NI = TKV // P  # 16 kv chunks
NHP = DQ // P  # 4 head-pairs
CQ_CH = CQ // P   # 8
CKV_CH = CKV // P  # 6
NCC = 2 * NHP     # 8 dc chunks in the gathered attention output

F32 = mybir.dt.float32
F32R = mybir.dt.float32r
EXP = mybir.ActivationFunctionType.Exp
ADD = mybir.AluOpType.add
MUL = mybir.AluOpType.mult

_compiled = None


def _build():
    nc = bacc.Bacc("TRN2", target_bir_lowering=False, debug=False,
                   num_devices=N_CORES)

    xqT = nc.dram_tensor("xqT", [CQ, TQ], F32R, kind="ExternalInput")
    xkvT = nc.dram_tensor("xkvT", [CKV, TKV], F32R, kind="ExternalInput")
    wqT = nc.dram_tensor("wqT", [CQ, DQ], F32R, kind="ExternalInput")
    wkT = nc.dram_tensor("wkT", [CKV, DQ], F32R, kind="ExternalInput")
    wvT = nc.dram_tensor("wvT", [CKV, DQ], F32R, kind="ExternalInput")
    # full-dc Wo slice for this core's output-channel half, dc rows in
    # gathered order (head-group 0 rows then head-group 1 rows)
    woT = nc.dram_tensor("woT", [2 * DQ, DO], F32R, kind="ExternalInput")
    onesc = nc.dram_tensor("onesc", [P, 8], F32R, kind="ExternalInput")
    out_ext = nc.dram_tensor("out", [DO, TQ], F32, kind="ExternalOutput")

    groups = [[2 * b, 2 * b + 1] for b in range(N_CORES // 2)]

    with tile.TileContext(nc) as tc:
        with (
            tc.tile_pool(name="weights", bufs=1) as wpool,
            tc.tile_pool(name="xload", bufs=1) as xpool,
            tc.tile_pool(name="stage", bufs=1) as stpool,
            tc.tile_pool(name="attn", bufs=1) as apool,
            tc.tile_pool(name="dram", bufs=1, space="DRAM") as dpool,
        ):
            # ---- resident weights ----
            wq_sb = wpool.tile([P, CQ_CH, DQ], F32R, tag="wq")
            wk_sb = wpool.tile([P, CKV_CH, DQ], F32R, tag="wk")
            wv_sb = wpool.tile([P, CKV_CH, DQ], F32R, tag="wv")
            wo_sb = wpool.tile([P, NCC, DO], F32R, tag="wo")
            nc.sync.dma_start(wv_sb[:], wvT.ap().rearrange("(n p) d -> p n d", p=P))
            nc.sync.dma_start(wk_sb[:], wkT.ap().rearrange("(n p) d -> p n d", p=P))
            nc.sync.dma_start(wq_sb[:], wqT.ap().rearrange("(n p) d -> p n d", p=P))
            ones_sb = wpool.tile([P, 8, 1], F32R, tag="ones")
            nc.sync.dma_start(ones_sb[:],
                              onesc.ap().rearrange("p (n o) -> p n o", o=1))
            nc.sync.dma_start(wo_sb[:], woT.ap().rearrange("(n p) d -> p n d", p=P))

            # ---- internal DRAM ----
            qT_d = dpool.tile([NHP, P, TQ], F32R, tag="qT_d")
            kT_d = dpool.tile([NHP, P, TKV], F32R, tag="kT_d")
            v_d = dpool.tile([TKV, 8 * 65], F32R, tag="v_d")
            ag_in = [dpool.tile([P, TQ], F32R, tag=f"agi{h}", name=f"agi{h}")
                     for h in range(NHP)]
            ag_out = [dpool.tile([2, P, TQ], F32R, tag=f"ago{h}",
                                 name=f"ago{h}")
                      for h in range(NHP)]
            o_acc = dpool.tile([DO, TQ], F32, tag="o_acc")

            xq_r = xqT.ap().rearrange("(n p) t -> p n t", p=P)
            xkv_r = xkvT.ap().rearrange("(n p) t -> p n t", p=P)
            v_r = v_d[:].rearrange("(n p) d -> p n d", p=P)

            # ================= projections =================
            with tc.tile_pool(name="psum_proj", bufs=4, space="PSUM") as ps_u:
                for tb in range(NTB):
                    ts_ = slice(tb * 512, (tb + 1) * 512)
                    xkv_t = []
                    for c in range(CKV_CH):
                        xkc = xpool.tile([P, 512], F32R, tag="xkv", bufs=8,
                                         name=f"xkv_{tb}_{c}")
                        nc.scalar.dma_start(xkc[:], xkv_r[:, c, ts_])
                        xkv_t.append(xkc)
                    xq_t = []
                    for c in range(CQ_CH):
                        xqc = xpool.tile([P, 512], F32R, tag="xq", bufs=10,
                                         name=f"xq_{tb}_{c}")
                        nc.scalar.dma_start(xqc[:], xq_r[:, c, ts_])
                        xq_t.append(xqc)

                    # V projection: v[t, dv] for the 4 t-chunks of this block
                    for s in range(4):
                        tc_i = tb * 4 + s
                        pv = ps_u.tile([P, 512], F32, tag="u")
                        for c in range(CKV_CH):
                            nc.tensor.matmul(
                                pv[:], xkv_t[c][:, s * P:(s + 1) * P],
                                wv_sb[:, c, :],
                                start=(c == 0), stop=(c == CKV_CH - 1))
                        vst = stpool.tile([P, 8, 65], F32R, tag="vstage",
                                          bufs=2)
                        nc.vector.tensor_copy(
                            vst[:, :, 0:64],
                            pv[:].rearrange("p (h d) -> p h d", d=64))
                        nc.vector.tensor_copy(vst[:, :, 64:65], ones_sb[:])
                        nc.sync.dma_start(v_d[tc_i * P:(tc_i + 1) * P, :],
                                          vst[:])

                    # K/Q projections into DRAM (channel-major, per head-pair)
                    for hp in range(NHP):
                        hs = slice(hp * P, (hp + 1) * P)
                        pk = ps_u.tile([P, 512], F32, tag="u")
                        for c in range(CKV_CH):
                            nc.tensor.matmul(
                                pk[:], wk_sb[:, c, hs], xkv_t[c][:],
                                start=(c == 0), stop=(c == CKV_CH - 1))
                        kst = stpool.tile([P, 512], F32R, tag="kqstage",
                                          bufs=2)
                        nc.vector.tensor_copy(kst[:], pk[:])
                        nc.sync.dma_start(kT_d[hp, :, ts_], kst[:])

                        pq = ps_u.tile([P, 512], F32, tag="u")
                        for c in range(CQ_CH):
                            nc.tensor.matmul(
                                pq[:], wq_sb[:, c, hs], xq_t[c][:],
                                start=(c == 0), stop=(c == CQ_CH - 1))
                        qst = stpool.tile([P, 512], F32R, tag="kqstage",
                                          bufs=2)
                        nc.vector.tensor_copy(qst[:], pq[:])
                        nc.sync.dma_start(qT_d[hp, :, ts_], qst[:])

            # ========== attention, with per-head-pair AllGather ==========
            with (
                tc.tile_pool(name="psum_s", bufs=2, space="PSUM") as ps_s,
                tc.tile_pool(name="psum_pv", bufs=2, space="PSUM") as ps_pv,
                tc.tile_pool(name="psum_op", bufs=2, space="PSUM") as ps_op,
            ):
                for hp in range(NHP):
                    kt = apool.tile([P, TKV], F32R, tag="kt", bufs=2)
                    nc.sync.dma_start(kt[:], kT_d[hp])
                    vh = apool.tile([P, NI, 130], F32R, tag="vh", bufs=2)
                    nc.sync.dma_start(vh[:], v_r[:, :, hp * 130:(hp + 1) * 130])
                    ao = apool.tile([P, TQ], F32R, tag="ao", bufs=2)
                    for j in range(NJ):
                        js = slice(j * 512, (j + 1) * 512)
                        qt = apool.tile([P, 512], F32R, tag="qt", bufs=3)
                        nc.sync.dma_start(qt[:], qT_d[hp, :, js])
                        acc_a = ps_pv.tile([P, 512], F32, tag="pv")
                        acc_b = ps_pv.tile([P, 512], F32, tag="pv")
                        for i in range(NI):
                            isl = slice(i * P, (i + 1) * P)
                            sc = ps_s.tile([P, 1024], F32, tag="sc")
                            nc.tensor.matmul(sc[:, 0:512], kt[0:64, isl],
                                             qt[0:64, :], start=True,
                                             stop=True)
                            nc.tensor.matmul(sc[:, 512:1024], kt[64:128, isl],
                                             qt[64:128, :], start=True,
                                             stop=True)
                            ex = stpool.tile([P, 1024], F32R, tag="ex", bufs=5)
                            nc.scalar.activation(ex[:], sc[:], EXP, scale=0.125)
                            nc.tensor.matmul(acc_a[0:65, :], vh[:, i, 0:65],
                                             ex[:, 0:512],
                                             start=(i == 0), stop=(i == NI - 1))
                            nc.tensor.matmul(acc_b[0:65, :], vh[:, i, 65:130],
                                             ex[:, 512:1024],
                                             start=(i == 0), stop=(i == NI - 1))
                        # normalize: ao[:, js] = acc[0:64] / acc[64]
                        for half, acc in ((0, acc_a), (1, acc_b)):
                            rec = stpool.tile([P, 512], F32, tag="rec", bufs=2)
                            nc.vector.reciprocal(rec[0:1, :], acc[64:65, :])
                            bc = stpool.tile([P, 512], F32, tag="bc", bufs=2)
                            nc.gpsimd.partition_broadcast(bc[0:64, :],
                                                          rec[0:1, :],
                                                          channels=64)
                            nc.vector.tensor_tensor(
                                ao[half * 64:(half + 1) * 64, js],
                                acc[0:64, :], bc[0:64, :], op=MUL)
                    # exchange this head-pair's attention output with the
                    # pair peer while later head-pairs keep computing
                    nc.sync.dma_start(ag_in[hp][:], ao[:])
                    nc.gpsimd.collective_compute(
                        "AllGather", mybir.AluOpType.bypass,
                        replica_groups=groups,
                        ins=[ag_in[hp].opt()], outs=[ag_out[hp].opt()])

                # ===== output projection (my half of the Wo columns) =====
                # head-pairs 0-2 are projected as soon as their AllGathers
                # land (overlapping the tail of attention); the last two
                # dc chunks are added after the final AllGather via a
                # DMA-accumulate, keeping the exposed tail small
                for j in range(NJ):
                    js = slice(j * 512, (j + 1) * 512)
                    rhs = []
                    for hp in range(NHP - 1):
                        for g in range(2):
                            aog = stpool.tile([P, 512], F32R, tag="aog",
                                              bufs=7, name=f"aog_{j}_{g}_{hp}")
                            nc.sync.dma_start(aog[:], ag_out[hp][g, :, js])
                            rhs.append(aog)
                    for do in range(DO // P):
                        po = ps_op.tile([P, 512], F32, tag="op")
                        for n in range(2 * (NHP - 1)):
                            cc = (n % 2) * NHP + n // 2
                            nc.tensor.matmul(
                                po[:], wo_sb[:, cc, do * P:(do + 1) * P],
                                rhs[n][:],
                                start=(n == 0), stop=(n == 2 * (NHP - 1) - 1))
                        ost = stpool.tile([P, 512], F32, tag="ost", bufs=2)
                        nc.vector.tensor_copy(ost[:], po[:])
                        nc.sync.dma_start(o_acc[do * P:(do + 1) * P, js],
                                          ost[:])
                for j in range(NJ):
                    js = slice(j * 512, (j + 1) * 512)
                    rhs = []
                    for g in range(2):
                        aog = stpool.tile([P, 512], F32R, tag="aog", bufs=7,
                                          name=f"aog3_{j}_{g}")
                        nc.sync.dma_start(aog[:], ag_out[NHP - 1][g, :, js])
                        rhs.append(aog)
                    for do in range(DO // P):
                        po = ps_op.tile([P, 512], F32, tag="op")
                        for g in range(2):
                            cc = g * NHP + NHP - 1
                            nc.tensor.matmul(
                                po[:], wo_sb[:, cc, do * P:(do + 1) * P],
                                rhs[g][:], start=(g == 0), stop=(g == 1))
                        ost = stpool.tile([P, 512], F32, tag="ost", bufs=2)
                        nc.vector.tensor_copy(ost[:], po[:])
                        nc.gpsimd.dma_start(o_acc[do * P:(do + 1) * P, js],
                                            ost[:], accum_op=ADD)
                        nc.sync.dma_start(out_ext[do * P:(do + 1) * P, js],
                                          o_acc[do * P:(do + 1) * P, js])

    nc.compile()
    return nc


def make_in_maps(q_tokens, kv_tokens, Wq, Wk, Wv, Wo):
    q_tokens = np.asarray(q_tokens, np.float32)
    kv_tokens = np.asarray(kv_tokens, np.float32)
    Wq = np.asarray(Wq, np.float32)
    Wk = np.asarray(Wk, np.float32)
    Wv = np.asarray(Wv, np.float32)
    Wo = np.asarray(Wo, np.float32)
    in_maps = []
    for c in range(N_CORES):
        b, hg = c // 2, c % 2
        sl = slice(hg * DQ, (hg + 1) * DQ)
        osl = slice(hg * DO, (hg + 1) * DO)
        in_maps.append({
            "xqT": np.ascontiguousarray(q_tokens[b].T),
            "xkvT": np.ascontiguousarray(kv_tokens[b].T),
            "wqT": np.ascontiguousarray(Wq[sl, :].T),
            "wkT": np.ascontiguousarray(Wk[sl, :].T),
            "wvT": np.ascontiguousarray(Wv[sl, :].T),
            # [dc, do-half] with dc rows in gathered (global head) order
            "woT": np.ascontiguousarray(Wo[osl, :].T),
            "onesc": np.ones((P, 8), np.float32),
        })
    return in_maps


def kernel(q_tokens, kv_tokens, Wq, Wk, Wv, Wo):
    global _compiled
    if _compiled is None:
        _compiled = _build()
    nc = _compiled

    in_maps = make_in_maps(q_tokens, kv_tokens, Wq, Wk, Wv, Wo)
    res = bass_utils.run_bass_kernel_spmd(nc, in_maps,
                                          core_ids=list(range(N_CORES)))
    B = 4
    out = np.empty((B, TQ, 2 * DO), np.float32)
    for c in range(N_CORES):
        b, hg = c // 2, c % 2
        out[b, :, hg * DO:(hg + 1) * DO] = res.results[c]["out"].T
    return out


# revision 20
# speedup vs baseline: 1.1199x; 1.1199x over previous
"""Cross-attention kernel for 8 Trainium2 NeuronCores (SPMD).

Problem: B=4, T_q=T_kv=2048, Q_DIM=1024, KV_DIM=768, H=16, DK=64, fp32.
  q = q_tokens @ Wq.T ; k = kv_tokens @ Wk.T ; v = kv_tokens @ Wv.T
  out = softmax(q k^T / sqrt(DK)) v @ Wo.T

Sharding (8 cores): core c handles batch b=c//2 and head-group hg=c%2
(8 heads, 512 of the 1024 q-dims).  After attention, the pair (2b, 2b+1)
AllGathers the per-head-group attention outputs (one collective per
head-pair chunk, overlapped with the remaining attention work), then each
core runs the output projection against ITS half of the Wo columns —
core c returns out[b, :, (c%2)*512:(c%2+1)*512] transposed.  The
rank-dependent output-channel split lives entirely in the host-side Wo
slice, so the device program is identical on all cores.

On-device layout is channel-major ("transposed") end-to-end: all host
inputs are pre-transposed so every matmul contraction dim lands on SBUF
partitions with no device-side transposes.  Softmax runs without
max-subtraction (scores are O(6) for randn inputs; exp is safe in fp32)
and the denominator comes free from an appended ones-column in V during
the PV matmul.  All matmuls run as float32r (full PE rate at
moving-dim 512); attention score matmuls for the two heads of a pair
run concurrently in the two 64-row halves of the PE array (row tiling).
"""

import numpy as np

import concourse.bacc as bacc
import concourse.mybir as mybir
import concourse.tile as tile
from concourse import bass_utils

N_CORES = 8
P = 128
TQ = 2048
TKV = 2048
CQ = 1024     # q_tokens channels
CKV = 768     # kv_tokens channels
DQ = 512      # per-core head-group q dims (8 heads x 64)
DO = 512      # per-core output channels (half of 1024)
NJ = 4        # 512-wide t-blocks
NTB = 4       # projection t-blocks
NI = TKV // P  # 16 kv chunks
NHP = DQ // P  # 4 head-pairs
CQ_CH = CQ // P   # 8
CKV_CH = CKV // P  # 6
NCC = 2 * NHP     # 8 dc chunks in the gathered attention output

F32 = mybir.dt.float32
F32R = mybir.dt.float32r
EXP = mybir.ActivationFunctionType.Exp
ADD = mybir.AluOpType.add
MUL = mybir.AluOpType.mult

_compiled = None


def _build():
    nc = bacc.Bacc("TRN2", target_bir_lowering=False, debug=False,
                   num_devices=N_CORES)

    xqT = nc.dram_tensor("xqT", [CQ, TQ], F32R, kind="ExternalInput")
    xkvT = nc.dram_tensor("xkvT", [CKV, TKV], F32R, kind="ExternalInput")
    wqT = nc.dram_tensor("wqT", [CQ, DQ], F32R, kind="ExternalInput")
    wkT = nc.dram_tensor("wkT", [CKV, DQ], F32R, kind="ExternalInput")
    wvT = nc.dram_tensor("wvT", [CKV, DQ], F32R, kind="ExternalInput")
    # full-dc Wo slice for this core's output-channel half, dc rows in
    # gathered order (head-group 0 rows then head-group 1 rows)
    woT = nc.dram_tensor("woT", [2 * DQ, DO], F32R, kind="ExternalInput")
    onesc = nc.dram_tensor("onesc", [P, 8], F32R, kind="ExternalInput")
    out_ext = nc.dram_tensor("out", [DO, TQ], F32, kind="ExternalOutput")

    groups = [[2 * b, 2 * b + 1] for b in range(N_CORES // 2)]

    with tile.TileContext(nc) as tc:
        with (
            tc.tile_pool(name="weights", bufs=1) as wpool,
            tc.tile_pool(name="xload", bufs=1) as xpool,
            tc.tile_pool(name="stage", bufs=1) as stpool,
            tc.tile_pool(name="attn", bufs=1) as apool,
            tc.tile_pool(name="dram", bufs=1, space="DRAM") as dpool,
        ):
            # ---- resident weights ----
            wq_sb = wpool.tile([P, CQ_CH, DQ], F32R, tag="wq")
            wk_sb = wpool.tile([P, CKV_CH, DQ], F32R, tag="wk")
            wv_sb = wpool.tile([P, CKV_CH, DQ], F32R, tag="wv")
            wo_sb = wpool.tile([P, NCC, DO], F32R, tag="wo")
            nc.sync.dma_start(wv_sb[:], wvT.ap().rearrange("(n p) d -> p n d", p=P))
            nc.sync.dma_start(wk_sb[:], wkT.ap().rearrange("(n p) d -> p n d", p=P))
            nc.sync.dma_start(wq_sb[:], wqT.ap().rearrange("(n p) d -> p n d", p=P))
            ones_sb = wpool.tile([P, 8, 1], F32R, tag="ones")
            nc.sync.dma_start(ones_sb[:],
                              onesc.ap().rearrange("p (n o) -> p n o", o=1))
            nc.sync.dma_start(wo_sb[:], woT.ap().rearrange("(n p) d -> p n d", p=P))

            # ---- internal DRAM ----
            qT_d = dpool.tile([NHP, P, TQ], F32R, tag="qT_d")
            kT_d = dpool.tile([NHP, P, TKV], F32R, tag="kT_d")
            v_d = dpool.tile([TKV, 8 * 65], F32R, tag="v_d")
            ag_in = [dpool.tile([P, TQ], F32R, tag=f"agi{h}", name=f"agi{h}")
                     for h in range(NHP)]
            ag_out = [dpool.tile([2, P, TQ], F32R, tag=f"ago{h}",
                                 name=f"ago{h}")
                      for h in range(NHP)]
            o_acc = dpool.tile([DO, TQ], F32, tag="o_acc")

            xq_r = xqT.ap().rearrange("(n p) t -> p n t", p=P)
            xkv_r = xkvT.ap().rearrange("(n p) t -> p n t", p=P)
            v_r = v_d[:].rearrange("(n p) d -> p n d", p=P)

            # ================= projections =================
            with tc.tile_pool(name="psum_proj", bufs=4, space="PSUM") as ps_u:
                warm = wpool.tile([P, 512], F32, tag="warm")
                nc.vector.memset(warm[:], 0.0)
                for w in range(24):
                    pw = ps_u.tile([P, 512], F32, tag="u", name=f"warm_{w}")
                    nc.tensor.matmul(pw[:], warm[:, 0:128], warm[:],
                                     start=True, stop=True)
                for tb in range(NTB):
                    ts_ = slice(tb * 512, (tb + 1) * 512)
                    xkv_t = []
                    for c in range(CKV_CH):
                        xkc = xpool.tile([P, 512], F32R, tag="xkv", bufs=8,
                                         name=f"xkv_{tb}_{c}")
                        nc.scalar.dma_start(xkc[:], xkv_r[:, c, ts_])
                        xkv_t.append(xkc)
                    xq_t = []
                    for c in range(CQ_CH):
                        xqc = xpool.tile([P, 512], F32R, tag="xq", bufs=10,
                                         name=f"xq_{tb}_{c}")
                        nc.scalar.dma_start(xqc[:], xq_r[:, c, ts_])
                        xq_t.append(xqc)

                    # V projection: v[t, dv] for the 4 t-chunks of this block
                    for s in range(4):
                        tc_i = tb * 4 + s
                        pv = ps_u.tile([P, 512], F32, tag="u")
                        for c in range(CKV_CH):
                            nc.tensor.matmul(
                                pv[:], xkv_t[c][:, s * P:(s + 1) * P],
                                wv_sb[:, c, :],
                                start=(c == 0), stop=(c == CKV_CH - 1))
                        vst = stpool.tile([P, 8, 65], F32R, tag="vstage",
                                          bufs=2)
                        nc.vector.tensor_copy(
                            vst[:, :, 0:64],
                            pv[:].rearrange("p (h d) -> p h d", d=64))
                        nc.vector.tensor_copy(vst[:, :, 64:65], ones_sb[:])
                        nc.sync.dma_start(v_d[tc_i * P:(tc_i + 1) * P, :],
                                          vst[:])

                    # K/Q projections into DRAM (channel-major, per head-pair)
                    for hp in range(NHP):
                        hs = slice(hp * P, (hp + 1) * P)
                        pk = ps_u.tile([P, 512], F32, tag="u")
                        for c in range(CKV_CH):
                            nc.tensor.matmul(
                                pk[:], wk_sb[:, c, hs], xkv_t[c][:],
                                start=(c == 0), stop=(c == CKV_CH - 1))
                        kst = stpool.tile([P, 512], F32R, tag="kqstage",
                                          bufs=2)
                        nc.vector.tensor_copy(kst[:], pk[:])
                        nc.sync.dma_start(kT_d[hp, :, ts_], kst[:])

                        pq = ps_u.tile([P, 512], F32, tag="u")
                        for c in range(CQ_CH):
                            nc.tensor.matmul(
                                pq[:], wq_sb[:, c, hs], xq_t[c][:],
                                start=(c == 0), stop=(c == CQ_CH - 1))
                        qst = stpool.tile([P, 512], F32R, tag="kqstage",
                                          bufs=2)
                        nc.vector.tensor_copy(qst[:], pq[:])
                        nc.sync.dma_start(qT_d[hp, :, ts_], qst[:])

            # ========== attention, with per-head-pair AllGather ==========
            with (
                tc.tile_pool(name="psum_s", bufs=2, space="PSUM") as ps_s,
                tc.tile_pool(name="psum_pv", bufs=2, space="PSUM") as ps_pv,
                tc.tile_pool(name="psum_op", bufs=2, space="PSUM") as ps_op,
            ):
                for hp in range(NHP):
                    kt = apool.tile([P, TKV], F32R, tag="kt", bufs=2)
                    nc.sync.dma_start(kt[:], kT_d[hp])
                    vh = apool.tile([P, NI, 130], F32R, tag="vh", bufs=2)
                    nc.sync.dma_start(vh[:], v_r[:, :, hp * 130:(hp + 1) * 130])
                    ao = apool.tile([P, TQ], F32R, tag="ao", bufs=2)
                    for j in range(NJ):
                        js = slice(j * 512, (j + 1) * 512)
                        qt = apool.tile([P, 512], F32R, tag="qt", bufs=3)
                        nc.sync.dma_start(qt[:], qT_d[hp, :, js])
                        acc_a = ps_pv.tile([P, 512], F32, tag="pv")
                        acc_b = ps_pv.tile([P, 512], F32, tag="pv")
                        for i in range(NI):
                            isl = slice(i * P, (i + 1) * P)
                            sc = ps_s.tile([P, 1024], F32, tag="sc")
                            nc.tensor.matmul(sc[:, 0:512], kt[0:64, isl],
                                             qt[0:64, :], start=True,
                                             stop=True)
                            nc.tensor.matmul(sc[:, 512:1024], kt[64:128, isl],
                                             qt[64:128, :], start=True,
                                             stop=True)
                            ex = stpool.tile([P, 1024], F32R, tag="ex", bufs=4)
                            nc.scalar.activation(ex[:], sc[:], EXP, scale=0.125)
                            nc.tensor.matmul(acc_a[0:65, :], vh[:, i, 0:65],
                                             ex[:, 0:512],
                                             start=(i == 0), stop=(i == NI - 1))
                            nc.tensor.matmul(acc_b[0:65, :], vh[:, i, 65:130],
                                             ex[:, 512:1024],
                                             start=(i == 0), stop=(i == NI - 1))
                        # evict accumulators to SBUF fast (frees the PSUM
                        # slots), then normalize from SBUF off the PE's
                        # critical path: ao[:, js] = acc[0:64] / acc[64]
                        for half, acc in ((0, acc_a), (1, acc_b)):
                            pvst = stpool.tile([P, 512], F32, tag="pvst",
                                               bufs=4,
                                               name=f"pvst_{hp}_{j}_{half}")
                            nc.vector.tensor_copy(pvst[0:65, :], acc[0:65, :])
                            rec = stpool.tile([P, 512], F32, tag="rec", bufs=2)
                            nc.vector.reciprocal(rec[0:1, :], pvst[64:65, :])
                            bc = stpool.tile([P, 512], F32, tag="bc", bufs=2)
                            nc.gpsimd.partition_broadcast(bc[0:64, :],
                                                          rec[0:1, :],
                                                          channels=64)
                            nc.vector.tensor_tensor(
                                ao[half * 64:(half + 1) * 64, js],
                                pvst[0:64, :], bc[0:64, :], op=MUL)
                    # exchange this head-pair's attention output with the
                    # pair peer while later head-pairs keep computing
                    nc.sync.dma_start(ag_in[hp][:], ao[:])
                    nc.gpsimd.collective_compute(
                        "AllGather", mybir.AluOpType.bypass,
                        replica_groups=groups,
                        ins=[ag_in[hp].opt()], outs=[ag_out[hp].opt()])

                # ===== output projection (my half of the Wo columns) =====
                # head-pairs 0-2 are projected as soon as their AllGathers
                # land (overlapping the tail of attention); the last two
                # dc chunks are added after the final AllGather via a
                # DMA-accumulate, keeping the exposed tail small
                for j in range(NJ):
                    js = slice(j * 512, (j + 1) * 512)
                    rhs = []
                    for hp in range(NHP - 1):
                        for g in range(2):
                            aog = stpool.tile([P, 512], F32R, tag="aog",
                                              bufs=7, name=f"aog_{j}_{g}_{hp}")
                            nc.sync.dma_start(aog[:], ag_out[hp][g, :, js])
                            rhs.append(aog)
                    for do in range(DO // P):
                        po = ps_op.tile([P, 512], F32, tag="op")
                        for n in range(2 * (NHP - 1)):
                            cc = (n % 2) * NHP + n // 2
                            nc.tensor.matmul(
                                po[:], wo_sb[:, cc, do * P:(do + 1) * P],
                                rhs[n][:],
                                start=(n == 0), stop=(n == 2 * (NHP - 1) - 1))
                        ost = stpool.tile([P, 512], F32, tag="ost", bufs=2)
                        nc.vector.tensor_copy(ost[:], po[:])
                        nc.sync.dma_start(o_acc[do * P:(do + 1) * P, js],
                                          ost[:])
                for j in range(NJ):
                    js = slice(j * 512, (j + 1) * 512)
                    rhs = []
                    for g in range(2):
                        aog = stpool.tile([P, 512], F32R, tag="aog", bufs=7,
                                          name=f"aog3_{j}_{g}")
                        nc.sync.dma_start(aog[:], ag_out[NHP - 1][g, :, js])
                        rhs.append(aog)
                    for do in range(DO // P):
                        po = ps_op.tile([P, 512], F32, tag="op")
                        for g in range(2):
                            cc = g * NHP + NHP - 1
                            nc.tensor.matmul(
                                po[:], wo_sb[:, cc, do * P:(do + 1) * P],
                                rhs[g][:], start=(g == 0), stop=(g == 1))
                        ost = stpool.tile([P, 512], F32, tag="ost", bufs=2)
                        nc.vector.tensor_copy(ost[:], po[:])
                        nc.gpsimd.dma_start(o_acc[do * P:(do + 1) * P, js],
                                            ost[:], accum_op=ADD)
                        nc.sync.dma_start(out_ext[do * P:(do + 1) * P, js],
                                          o_acc[do * P:(do + 1) * P, js])

    nc.compile()
    return nc


def make_in_maps(q_tokens, kv_tokens, Wq, Wk, Wv, Wo):
    q_tokens = np.asarray(q_tokens, np.float32)
    kv_tokens = np.asarray(kv_tokens, np.float32)
    Wq = np.asarray(Wq, np.float32)
    Wk = np.asarray(Wk, np.float32)
    Wv = np.asarray(Wv, np.float32)
    Wo = np.asarray(Wo, np.float32)
    in_maps = []
    for c in range(N_CORES):
        b, hg = c // 2, c % 2
        sl = slice(hg * DQ, (hg + 1) * DQ)
        osl = slice(hg * DO, (hg + 1) * DO)
        in_maps.append({
            "xqT": np.ascontiguousarray(q_tokens[b].T),
            "xkvT": np.ascontiguousarray(kv_tokens[b].T),
            "wqT": np.ascontiguousarray(Wq[sl, :].T),
            "wkT": np.ascontiguousarray(Wk[sl, :].T),
            "wvT": np.ascontiguousarray(Wv[sl, :].T),
            # [dc, do-half] with dc rows in gathered (global head) order
            "woT": np.ascontiguousarray(Wo[osl, :].T),
            "onesc": np.ones((P, 8), np.float32),
        })
    return in_maps


def kernel(q_tokens, kv_tokens, Wq, Wk, Wv, Wo):
    global _compiled
    if _compiled is None:
        _compiled = _build()
    nc = _compiled

    in_maps = make_in_maps(q_tokens, kv_tokens, Wq, Wk, Wv, Wo)
    res = bass_utils.run_bass_kernel_spmd(nc, in_maps,
                                          core_ids=list(range(N_CORES)))
    B = 4
    out = np.empty((B, TQ, 2 * DO), np.float32)
    for c in range(N_CORES):
        b, hg = c // 2, c % 2
        out[b, :, hg * DO:(hg + 1) * DO] = res.results[c]["out"].T
    return out


# revision 24
# speedup vs baseline: 1.2598x; 1.1249x over previous
"""Cross-attention kernel for 8 Trainium2 NeuronCores (SPMD).

Problem: B=4, T_q=T_kv=2048, Q_DIM=1024, KV_DIM=768, H=16, DK=64, fp32.
  q = q_tokens @ Wq.T ; k = kv_tokens @ Wk.T ; v = kv_tokens @ Wv.T
  out = softmax(q k^T / sqrt(DK)) v @ Wo.T

Sharding (8 cores): core c handles batch b=c//2 and head-group hg=c%2
(8 heads, 512 of the 1024 q-dims).  After attention, the pair (2b, 2b+1)
AllGathers the per-head-group attention outputs (one collective per
head-pair chunk, overlapped with the remaining attention work), then each
core runs the output projection against ITS half of the Wo columns —
core c returns out[b, :, (c%2)*512:(c%2+1)*512] transposed.  The
rank-dependent output-channel split lives entirely in the host-side Wo
slice, so the device program is identical on all cores.

On-device layout is channel-major ("transposed") end-to-end: all host
inputs are pre-transposed so every matmul contraction dim lands on SBUF
partitions with no device-side transposes.  Softmax runs without
max-subtraction (scores are O(6) for randn inputs; exp is safe in fp32)
and the denominator comes free from an appended ones-column in V during
the PV matmul.  All matmuls run as float32r (full PE rate at
moving-dim 512); attention score matmuls for the two heads of a pair
run concurrently in the two 64-row halves of the PE array (row tiling).
"""

import numpy as np

import concourse.bacc as bacc
import concourse.mybir as mybir
import concourse.tile as tile
from concourse import bass_utils

N_CORES = 8
P = 128
TQ = 2048
TKV = 2048
CQ = 1024     # q_tokens channels
CKV = 768     # kv_tokens channels
DQ = 512      # per-core head-group q dims (8 heads x 64)
DO = 512      # per-core output channels (half of 1024)
NJ = 4        # 512-wide t-blocks
NTB = 4       # projection t-blocks
NI = TKV // P  # 16 kv chunks
NHP = DQ // P  # 4 head-pairs
CQ_CH = CQ // P   # 8
CKV_CH = CKV // P  # 6
NCC = 2 * NHP     # 8 dc chunks in the gathered attention output

F32 = mybir.dt.float32
F32R = mybir.dt.float32r
EXP = mybir.ActivationFunctionType.Exp
ADD = mybir.AluOpType.add
MUL = mybir.AluOpType.mult

_compiled = None


def _build():
    nc = bacc.Bacc("TRN2", target_bir_lowering=False, debug=False,
                   num_devices=N_CORES)

    xqT = nc.dram_tensor("xqT", [CQ, TQ], F32R, kind="ExternalInput")
    xkvT = nc.dram_tensor("xkvT", [CKV, TKV], F32R, kind="ExternalInput")
    wqT = nc.dram_tensor("wqT", [CQ, DQ], F32R, kind="ExternalInput")
    wkT = nc.dram_tensor("wkT", [CKV, DQ], F32R, kind="ExternalInput")
    wvT = nc.dram_tensor("wvT", [CKV, DQ], F32R, kind="ExternalInput")
    # full-dc Wo slice for this core's output-channel half, dc rows in
    # gathered order (head-group 0 rows then head-group 1 rows)
    woT = nc.dram_tensor("woT", [2 * DQ, DO], F32R, kind="ExternalInput")
    onesc = nc.dram_tensor("onesc", [P, 8], F32R, kind="ExternalInput")
    out_ext = nc.dram_tensor("out", [DO, TQ], F32, kind="ExternalOutput")

    groups = [[2 * b, 2 * b + 1] for b in range(N_CORES // 2)]

    with tile.TileContext(nc) as tc:
        with (
            tc.tile_pool(name="weights", bufs=1) as wpool,
            tc.tile_pool(name="xload", bufs=1) as xpool,
            tc.tile_pool(name="stage", bufs=1) as stpool,
            tc.tile_pool(name="attn", bufs=1) as apool,
            tc.tile_pool(name="dram", bufs=1, space="DRAM") as dpool,
        ):
            # ---- resident weights ----
            wq_sb = wpool.tile([P, CQ_CH, DQ], F32R, tag="wq")
            wk_sb = wpool.tile([P, CKV_CH, DQ], F32R, tag="wk")
            wv_sb = wpool.tile([P, CKV_CH, DQ], F32R, tag="wv")
            wo_sb = wpool.tile([P, NCC, DO], F32R, tag="wo")
            nc.sync.dma_start(wv_sb[:], wvT.ap().rearrange("(n p) d -> p n d", p=P))
            nc.sync.dma_start(wk_sb[:], wkT.ap().rearrange("(n p) d -> p n d", p=P))
            nc.sync.dma_start(wq_sb[:], wqT.ap().rearrange("(n p) d -> p n d", p=P))
            ones_sb = wpool.tile([P, 8, 1], F32R, tag="ones")
            nc.sync.dma_start(ones_sb[:],
                              onesc.ap().rearrange("p (n o) -> p n o", o=1))
            nc.sync.dma_start(wo_sb[:], woT.ap().rearrange("(n p) d -> p n d", p=P))

            # ---- internal DRAM ----
            qT_d = dpool.tile([NHP, P, TQ], F32R, tag="qT_d")
            kT_d = dpool.tile([NHP, P, TKV], F32R, tag="kT_d")
            v_d = dpool.tile([TKV, 8 * 65], F32R, tag="v_d")
            ag_in = [dpool.tile([P, TQ], F32R, tag=f"agi{h}", name=f"agi{h}")
                     for h in range(NHP)]
            ag_out = [dpool.tile([2, P, TQ], F32R, tag=f"ago{h}",
                                 name=f"ago{h}")
                      for h in range(NHP - 1)]
            ag_out4 = [dpool.tile([2, P, TQ // 2], F32R, tag=f"ago4{h}",
                                  name=f"ago4{h}")
                       for h in range(2)]
            ag_in4 = [dpool.tile([P, TQ // 2], F32R, tag=f"agi4{h}",
                                 name=f"agi4{h}")
                      for h in range(2)]

            xq_r = xqT.ap().rearrange("(n p) t -> p n t", p=P)
            xkv_r = xkvT.ap().rearrange("(n p) t -> p n t", p=P)
            v_r = v_d[:].rearrange("(n p) d -> p n d", p=P)

            # ================= projections =================
            with tc.tile_pool(name="psum_proj", bufs=4, space="PSUM") as ps_u:
                warm = wpool.tile([P, P], F32, tag="warm")
                nc.vector.memset(warm[:], 0.0)
                for w in range(24):
                    pw = ps_u.tile([P, 512], F32, tag="u", name=f"warm_{w}")
                    nc.tensor.matmul(pw[:, 0:P], warm[:], warm[:],
                                     start=True, stop=True)
                for tb in range(NTB):
                    ts_ = slice(tb * 512, (tb + 1) * 512)
                    xkv_t = []
                    for c in range(CKV_CH):
                        xkc = xpool.tile([P, 512], F32R, tag="xkv", bufs=7,
                                         name=f"xkv_{tb}_{c}")
                        nc.scalar.dma_start(xkc[:], xkv_r[:, c, ts_])
                        xkv_t.append(xkc)
                    xq_t = []
                    for c in range(CQ_CH):
                        xqc = xpool.tile([P, 512], F32R, tag="xq", bufs=8,
                                         name=f"xq_{tb}_{c}")
                        nc.scalar.dma_start(xqc[:], xq_r[:, c, ts_])
                        xq_t.append(xqc)

                    # V projection: v[t, dv] for the 4 t-chunks of this block
                    for s in range(4):
                        tc_i = tb * 4 + s
                        pv = ps_u.tile([P, 512], F32, tag="u")
                        for c in range(CKV_CH):
                            nc.tensor.matmul(
                                pv[:], xkv_t[c][:, s * P:(s + 1) * P],
                                wv_sb[:, c, :],
                                start=(c == 0), stop=(c == CKV_CH - 1))
                        vst = stpool.tile([P, 8, 65], F32R, tag="vstage",
                                          bufs=2)
                        nc.vector.tensor_copy(
                            vst[:, :, 0:64],
                            pv[:].rearrange("p (h d) -> p h d", d=64))
                        nc.vector.tensor_copy(vst[:, :, 64:65], ones_sb[:])
                        nc.sync.dma_start(v_d[tc_i * P:(tc_i + 1) * P, :],
                                          vst[:])

                    # K/Q projections into DRAM (channel-major, per head-pair)
                    for hp in range(NHP):
                        hs = slice(hp * P, (hp + 1) * P)
                        pk = ps_u.tile([P, 512], F32, tag="u")
                        for c in range(CKV_CH):
                            nc.tensor.matmul(
                                pk[:], wk_sb[:, c, hs], xkv_t[c][:],
                                start=(c == 0), stop=(c == CKV_CH - 1))
                        kst = stpool.tile([P, 512], F32R, tag="kqstage",
                                          bufs=2)
                        nc.vector.tensor_copy(kst[:], pk[:])
                        nc.sync.dma_start(kT_d[hp, :, ts_], kst[:])

                        pq = ps_u.tile([P, 512], F32, tag="u")
                        for c in range(CQ_CH):
                            nc.tensor.matmul(
                                pq[:], wq_sb[:, c, hs], xq_t[c][:],
                                start=(c == 0), stop=(c == CQ_CH - 1))
                        qst = stpool.tile([P, 512], F32R, tag="kqstage",
                                          bufs=2)
                        nc.vector.tensor_copy(qst[:], pq[:])
                        nc.sync.dma_start(qT_d[hp, :, ts_], qst[:])

            # ========== attention, with per-head-pair AllGather ==========
            with (
                tc.tile_pool(name="psum_s", bufs=2, space="PSUM") as ps_s,
                tc.tile_pool(name="psum_pv", bufs=2, space="PSUM") as ps_pv,
                tc.tile_pool(name="psum_op", bufs=2, space="PSUM") as ps_op,
            ):
                for hp in range(NHP):
                    kt = apool.tile([P, TKV], F32R, tag="kt", bufs=2)
                    nc.sync.dma_start(kt[:], kT_d[hp])
                    vh = apool.tile([P, NI, 130], F32R, tag="vh", bufs=2)
                    nc.sync.dma_start(vh[:], v_r[:, :, hp * 130:(hp + 1) * 130])
                    ao = apool.tile([P, TQ], F32R, tag="ao", bufs=1)
                    for j in range(NJ):
                        js = slice(j * 512, (j + 1) * 512)
                        qt = apool.tile([P, 512], F32R, tag="qt", bufs=2)
                        nc.sync.dma_start(qt[:], qT_d[hp, :, js])
                        acc_a = ps_pv.tile([P, 512], F32, tag="pv")
                        acc_b = ps_pv.tile([P, 512], F32, tag="pv")
                        for i in range(NI):
                            isl = slice(i * P, (i + 1) * P)
                            sc = ps_s.tile([P, 1024], F32, tag="sc")
                            nc.tensor.matmul(sc[:, 0:512], kt[0:64, isl],
                                             qt[0:64, :], start=True,
                                             stop=True)
                            nc.tensor.matmul(sc[:, 512:1024], kt[64:128, isl],
                                             qt[64:128, :], start=True,
                                             stop=True)
                            ex = stpool.tile([P, 1024], F32R, tag="ex", bufs=4)
                            nc.scalar.activation(ex[:], sc[:], EXP, scale=0.125)
                            nc.tensor.matmul(acc_a[0:65, :], vh[:, i, 0:65],
                                             ex[:, 0:512],
                                             start=(i == 0), stop=(i == NI - 1))
                            nc.tensor.matmul(acc_b[0:65, :], vh[:, i, 65:130],
                                             ex[:, 512:1024],
                                             start=(i == 0), stop=(i == NI - 1))
                        # evict accumulators to SBUF fast (frees the PSUM
                        # slots), then normalize from SBUF off the PE's
                        # critical path: ao[:, js] = acc[0:64] / acc[64]
                        for half, acc in ((0, acc_a), (1, acc_b)):
                            pvst = stpool.tile([P, 512], F32, tag="pvst",
                                               bufs=3,
                                               name=f"pvst_{hp}_{j}_{half}")
                            nc.vector.tensor_copy(pvst[0:65, :], acc[0:65, :])
                            rec = stpool.tile([P, 512], F32, tag="rec", bufs=2)
                            nc.vector.reciprocal(rec[0:1, :], pvst[64:65, :])
                            bc = stpool.tile([P, 512], F32, tag="bc", bufs=2)
                            nc.gpsimd.partition_broadcast(bc[0:64, :],
                                                          rec[0:1, :],
                                                          channels=64)
                            nc.vector.tensor_tensor(
                                ao[half * 64:(half + 1) * 64, js],
                                pvst[0:64, :], bc[0:64, :], op=MUL)
                    # exchange this head-pair's attention output with the
                    # pair peer while later head-pairs keep computing; the
                    # last head-pair goes in two halves so the output
                    # projection can start before its second half lands
                    if hp < NHP - 1:
                        nc.sync.dma_start(ag_in[hp][:], ao[:])
                        nc.gpsimd.collective_compute(
                            "AllGather", mybir.AluOpType.bypass,
                            replica_groups=groups,
                            ins=[ag_in[hp].opt()], outs=[ag_out[hp].opt()])
                    else:
                        for hl in range(2):
                            hsl = slice(hl * 1024, (hl + 1) * 1024)
                            nc.sync.dma_start(ag_in4[hl][:], ao[:, hsl])
                            nc.gpsimd.collective_compute(
                                "AllGather", mybir.AluOpType.bypass,
                                replica_groups=groups,
                                ins=[ag_in4[hl].opt()],
                                outs=[ag_out4[hl].opt()])

                # ===== output projection (my half of the Wo columns) =====
                # cc chunk order keeps the last head-pair's chunks last in
                # each accumulation so j-blocks 0-1 can start right after
                # the first half-AllGather of head-pair 3
                for j in range(NJ):
                    js = slice(j * 512, (j + 1) * 512)
                    rhs = []
                    for n in range(NCC):
                        g, hp = n % 2, n // 2
                        aog = stpool.tile([P, 512], F32R, tag="aog",
                                          bufs=9, name=f"aog_{j}_{g}_{hp}")
                        if hp < NHP - 1:
                            nc.sync.dma_start(aog[:], ag_out[hp][g, :, js])
                        else:
                            nc.sync.dma_start(
                                aog[:],
                                ag_out4[j // 2][g, :,
                                                (j % 2) * 512:(j % 2 + 1) * 512])
                        rhs.append(aog)
                    for do in range(DO // P):
                        po = ps_op.tile([P, 512], F32, tag="op")
                        for n in range(NCC):
                            cc = (n % 2) * NHP + n // 2
                            nc.tensor.matmul(
                                po[:], wo_sb[:, cc, do * P:(do + 1) * P],
                                rhs[n][:],
                                start=(n == 0), stop=(n == NCC - 1))
                        ost = stpool.tile([P, 512], F32, tag="ost", bufs=2)
                        nc.vector.tensor_copy(ost[:], po[:])
                        nc.sync.dma_start(out_ext[do * P:(do + 1) * P, js],
                                          ost[:])

    nc.compile()
    return nc


def make_in_maps(q_tokens, kv_tokens, Wq, Wk, Wv, Wo):
    q_tokens = np.asarray(q_tokens, np.float32)
    kv_tokens = np.asarray(kv_tokens, np.float32)
    Wq = np.asarray(Wq, np.float32)
    Wk = np.asarray(Wk, np.float32)
    Wv = np.asarray(Wv, np.float32)
    Wo = np.asarray(Wo, np.float32)
    in_maps = []
    for c in range(N_CORES):
        b, hg = c // 2, c % 2
        sl = slice(hg * DQ, (hg + 1) * DQ)
        osl = slice(hg * DO, (hg + 1) * DO)
        in_maps.append({
            "xqT": np.ascontiguousarray(q_tokens[b].T),
            "xkvT": np.ascontiguousarray(kv_tokens[b].T),
            "wqT": np.ascontiguousarray(Wq[sl, :].T),
            "wkT": np.ascontiguousarray(Wk[sl, :].T),
            "wvT": np.ascontiguousarray(Wv[sl, :].T),
            # [dc, do-half] with dc rows in gathered (global head) order
            "woT": np.ascontiguousarray(Wo[osl, :].T),
            "onesc": np.ones((P, 8), np.float32),
        })
    return in_maps


def kernel(q_tokens, kv_tokens, Wq, Wk, Wv, Wo):
    global _compiled
    if _compiled is None:
        _compiled = _build()
    nc = _compiled

    in_maps = make_in_maps(q_tokens, kv_tokens, Wq, Wk, Wv, Wo)
    res = bass_utils.run_bass_kernel_spmd(nc, in_maps,
                                          core_ids=list(range(N_CORES)))
    B = 4
    out = np.empty((B, TQ, 2 * DO), np.float32)
    for c in range(N_CORES):
        b, hg = c // 2, c % 2
        out[b, :, hg * DO:(hg + 1) * DO] = res.results[c]["out"].T
    return out
